# revision 1
# baseline (speedup 1.0000x reference)
"""CompartmentAwareNormalization Trainium2 kernel.

Math (reference, per token t with d_model D=1024, NC=5 compartments):
    mu, var   = stats(x_t)                       (biased, over D)
    n_t       = (x_t - mu) * rsqrt(var + eps)
    y_t       = (n_t * gamma[c] + beta[c]) * scale[c],   c = cid_t
    out_t     = y_t + y_t @ W.T + b

Rewrite used here (all ids are valid: 0 <= cid < 5 per the input spec):
    Gp  = gamma * scale[:, None]            [5, D]
    Bp  = beta  * scale[:, None]            [5, D]
    W2  = W.T + I                           [D, D]   (folds the +y residual)
    A   = Bp @ W2 + b                       [5, D]
    GW  = Gp @ W2                           [5, D]
    z_t = n_t * Gp[c]                       (so y_t = z_t + Bp[c])
    out_t = z_t @ W2 + A[c]

On device everything runs in a d-major ("transposed") layout: x is
pre-transposed on the host to xT[D, T].  Then with per-token istd_t and
nmi_t = -mu_t * istd_t:
    zT = xT * PP + QQ,  PP[d,t] = Gp[c_t,d]*istd_t,  QQ[d,t] = Gp[c_t,d]*nmi_t
    outT[e,t] = sum_d W2[d,e] * (xT*PP)[d,t]            (main matmuls)
              + sum_c GW[c,e] * (onehot[c,t]*nmi_t)     (QQ via table gather)
              + sum_c A[c,e]  *  onehot[c,t]            (A   via table gather)
The last two are one K=10 matmul with lhsT = [GW; A] and
rhs = [onehot*nmi; onehot].  PP comes from a K=5 matmul of Gp against
onehot*istd.  Stats (sum x, sum x^2) come from ones-vector matmuls.

Sharding: data-parallel over tokens; core c takes tokens
[c*4096, (c+1)*4096).  W2 and the small tables are replicated.
"""

import numpy as np
from contextlib import ExitStack

import concourse.bass as bass
import concourse.bacc as bacc
import concourse.tile as tile
from concourse import mybir
from concourse import bass_utils

B, S, D = 4, 8192, 1024
NC = 5
EPS = 1e-5
NCORES = 8
T = B * S                  # 32768 tokens total
TPC = T // NCORES          # 4096 tokens per core
TB = 512                   # tokens per device block
NBLK = TPC // TB
NKC = D // 128             # contraction chunks
NET = D // 128             # output e-tiles
F32 = mybir.dt.float32

# matmul compute dtype: bf16 streams at 1 cycle/row (fp32 is 4 cycles/row;
# float32r needs every producer to round-to-FP32R, which DMA cannot do)
MM_MAIN = mybir.dt.float16
MM_AUX = mybir.dt.bfloat16


def _v(ap, dt):
    return ap.bitcast(dt) if ap.dtype != dt else ap


def _build_nc(repeat=1):
    # Bacc (not plain Bass): its compile() splits multi-semaphore waits into
    # event-semaphore instructions — TRN2 engine instructions can carry at
    # most one sync wait, and Tile freely emits more than one.
    nc = bacc.Bacc()
    tdt = MM_MAIN                        # storage dtype of W2/tables/zt/oh/gp

    xT = nc.declare_dram_parameter("xT", [D, TPC], F32, False)
    oh = nc.declare_dram_parameter("oh", [2 * NC, TPC], tdt, False)
    w2 = nc.declare_dram_parameter("w2", [D, D], tdt, False)
    gp = nc.declare_dram_parameter("gp", [NC, D], tdt, False)
    t10 = nc.declare_dram_parameter("t10", [2 * NC, D], tdt, False)
    outT = nc.declare_dram_parameter("outT", [D, TPC], F32, True)

    with tile.TileContext(nc) as tc, ExitStack() as ctx:
        singles = ctx.enter_context(tc.tile_pool(name="singles", bufs=1))
        xpool = ctx.enter_context(tc.tile_pool(name="xpool", bufs=3))
        sqpool = ctx.enter_context(tc.tile_pool(name="sqpool", bufs=3))
        zpool = ctx.enter_context(tc.tile_pool(name="zpool", bufs=3))
        stat = ctx.enter_context(tc.tile_pool(name="stat", bufs=1))
        rpool = ctx.enter_context(tc.tile_pool(name="rpool", bufs=2))
        osb = ctx.enter_context(tc.tile_pool(name="osb", bufs=3))
        spsum = ctx.enter_context(tc.tile_pool(name="spsum", bufs=1, space="PSUM"))
        ppsum = ctx.enter_context(tc.tile_pool(name="ppsum", bufs=3, space="PSUM"))
        opsum = ctx.enter_context(tc.tile_pool(name="opsum", bufs=3, space="PSUM"))

        # resident tables; W2 split into per-chunk DMAs so the first block's
        # input load isn't queued behind one monolithic 2 MB transfer
        w2sb = singles.tile([128, NKC, D], tdt)          # [p, k, e], d = 128k+p
        w2_r = w2.rearrange("(k p) e -> p k e", p=128)
        for k in range(NKC):
            nc.sync.dma_start(out=w2sb[:, k, :], in_=w2_r[:, k, :])
        gpsb = singles.tile([NC, NKC, 128], tdt)         # [c, k, d_in]
        nc.sync.dma_start(out=gpsb, in_=gp.rearrange("c (k d) -> c k d", d=128))
        t10sb = singles.tile([2 * NC, NET, 128], tdt)    # [c, j, e_in]
        nc.sync.dma_start(out=t10sb, in_=t10.rearrange("c (j e) -> c j e", e=128))
        ohsb = singles.tile([2 * NC, TPC], tdt)
        nc.sync.dma_start(out=ohsb, in_=oh[:, :])
        # M=5 ones so the stats matmuls write [5, TB] (NC identical rows);
        # downstream [5, TB] stat math then needs no partition broadcast
        ones = singles.tile([128, NC], tdt)
        nc.vector.memset(ones, 1.0)
        eps_ap = singles.tile([NC, 1], F32)
        nc.vector.memset(eps_ap, EPS)

        xT_r = xT.rearrange("(k p) t -> p k t", p=128)
        outT_r = outT.rearrange("(j p) t -> p j t", p=128)

        # timing-only outer loop (repeat=1 for normal runs; the kernel is a
        # pure function of its inputs so re-running it is idempotent)
        rep_ctx = tc.For_i(0, repeat, 1) if repeat > 1 else None
        if rep_ctx is not None:
            ctx.enter_context(rep_ctx)

        for blk in range(NBLK):
            tsl = slice(blk * TB, (blk + 1) * TB)

            xt = xpool.tile([128, NKC, TB], F32)
            nc.sync.dma_start(out=xt, in_=xT_r[:, :, tsl])

            # bf16 views of x for the stats matmuls: square on ACT, plain
            # cast on the otherwise-idle GPSIMD engine
            xsq = sqpool.tile([128, NKC, TB], tdt)
            nc.scalar.square(out=xsq, in_=xt)
            xtb = sqpool.tile([128, NKC, TB], tdt)
            nc.gpsimd.tensor_copy(xtb, xt)

            # token stats via ones-matmuls: s1 = sum_d x, s2 = sum_d x^2
            s1 = spsum.tile([NC, TB], F32)
            s2 = spsum.tile([NC, TB], F32)
            for k in range(NKC):
                nc.tensor.matmul(s1, ones, xtb[:, k, :],
                                 start=(k == 0), stop=(k == NKC - 1))
            for k in range(NKC):
                nc.tensor.matmul(s2, ones, xsq[:, k, :],
                                 start=(k == 0), stop=(k == NKC - 1))

            # mu = s1/D ; var = (s2 - D*mu^2)/D ; istd = 1/sqrt(var+eps)
            # (kept on DVE so matmuls waiting on psum slots only wait on DVE)
            neg_mu = stat.tile([NC, TB], F32)
            nc.vector.tensor_scalar_mul(neg_mu, s1, -1.0 / D)
            t1 = stat.tile([NC, TB], F32)
            nc.vector.tensor_mul(t1, s1, neg_mu)           # -D*mu^2
            t2 = stat.tile([NC, TB], F32)
            nc.vector.tensor_add(t2, s2, t1)               # D*var
            std = stat.tile([NC, TB], F32)
            nc.scalar.activation(out=std, in_=t2,
                                 func=mybir.ActivationFunctionType.Sqrt,
                                 bias=eps_ap[:, :], scale=1.0 / D)
            istd = stat.tile([NC, TB], F32)
            nc.vector.reciprocal(istd, std)
            nmi = stat.tile([NC, TB], F32)                 # -mu*istd
            nc.vector.tensor_mul(nmi, neg_mu, istd)

            r5 = rpool.tile([NC, TB], tdt)                 # onehot * istd
            nc.vector.tensor_mul(r5, ohsb[0:NC, tsl], istd)
            r10 = rpool.tile([2 * NC, TB], tdt)            # [onehot*nmi; onehot]
            nc.vector.tensor_mul(r10[0:NC], ohsb[0:NC, tsl], nmi)
            # engine ops can't write starting at partition 5; DMA can
            nc.sync.dma_start(out=r10[NC:], in_=ohsb[NC:, tsl])

            # zt[d,t] = x[d,t] * Gp[cid_t, d] * istd_t
            zt = zpool.tile([128, NKC, TB], tdt)
            for k in range(NKC):
                pp = ppsum.tile([128, TB], F32)
                nc.tensor.matmul(pp, gpsb[:, k, :], r5)
                nc.vector.tensor_mul(zt[:, k, :], xt[:, k, :], pp)

            # outT[e,t] = sum_k W2[k-chunk,e].T @ zt[k] + [GW;A].T @ r10
            for j in range(NET):
                op = opsum.tile([128, TB], F32)
                for k in range(NKC):
                    nc.tensor.matmul(op, w2sb[:, k, j * 128:(j + 1) * 128],
                                     zt[:, k, :],
                                     start=(k == 0), stop=False)
                nc.tensor.matmul(op, t10sb[:, j, :], r10,
                                 start=False, stop=True)
                ot = osb.tile([128, TB], F32)
                nc.vector.tensor_copy(ot, op)
                nc.sync.dma_start(out=outT_r[:, j, tsl], in_=ot)

    nc.compile()
    return nc


_CACHE = {}


def _get_nc(repeat=1):
    key = ("nc", repeat)
    if key not in _CACHE:
        _CACHE[key] = _build_nc(repeat)
    return _CACHE[key]


def _prep_host(x, compartment_ids, gamma, beta, scale, W, b):
    tnp = mybir.dt.np(MM_MAIN)

    x = np.asarray(x, dtype=np.float32)
    ids = np.asarray(compartment_ids).reshape(T).astype(np.int64)
    gamma = np.asarray(gamma, dtype=np.float32)
    beta = np.asarray(beta, dtype=np.float32)
    scale = np.asarray(scale, dtype=np.float32)
    W = np.asarray(W, dtype=np.float32)
    b = np.asarray(b, dtype=np.float32)

    Gp = gamma * scale[:, None]
    Bp = beta * scale[:, None]
    W2 = W.T.astype(np.float32) + np.eye(D, dtype=np.float32)
    A = (Bp @ W2 + b).astype(tnp)
    GW = (Gp @ W2).astype(tnp)
    W2 = np.ascontiguousarray(W2).astype(tnp)

    Gp_t = Gp.astype(tnp)
    onehot = (ids[None, :] == np.arange(NC)[:, None]).astype(tnp)
    oh10 = np.concatenate([onehot, onehot], axis=0)        # [10, T]
    T10 = np.concatenate([GW, A], axis=0)                  # [10, D]

    x3 = x.reshape(NCORES, TPC, D)
    xT_all = np.ascontiguousarray(x3.transpose(0, 2, 1))    # [cores, D, TPC]

    in_maps = []
    for c in range(NCORES):
        in_maps.append({
            "xT": xT_all[c],
            "oh": np.ascontiguousarray(oh10[:, c * TPC:(c + 1) * TPC]),
            "w2": W2,
            "gp": Gp_t,
            "t10": T10,
        })
    return in_maps


def _assemble(results):
    outs = [results[c]["outT"] for c in range(NCORES)]      # each [D, TPC]
    out = np.stack(outs, axis=0).transpose(0, 2, 1)         # [cores, TPC, D]
    return np.ascontiguousarray(out).reshape(B, S, D).astype(np.float32)


def _run(inputs, trace=False, repeat=1):
    nc = _get_nc(repeat)
    in_maps = _prep_host(**inputs)
    res = bass_utils.run_bass_kernel_spmd(
        nc, in_maps, list(range(NCORES)), trace=trace)
    return _assemble(res.results), res


def kernel(**inputs):
    out, _ = _run(inputs)
    return out

